# revision 11
# baseline (speedup 1.0000x reference)
"""Canny-edge pipeline for Trainium2, 8-core data-parallel (4 images/core).

Layout: H on partitions in 5 overlapping row-segments of 128 rows
(stride 124, 2-row halo each side), W on the free dim with 2-col zero pad
(segment width 516, data cols 2..513).

Per channel:
  hB   = 5-tap Gaussian along W            (DVE/GPSIMD shifted-AP ops)
  blur = 5-tap Gaussian along H of hB      (PE banded matmul, psum->ACT copy)
  hGx  = [1,0,-1] along W of blur          (DVE)
  hGy  = [1,2,1]  along W of blur          (DVE)
  gx   = [1,2,1]  along H of hGx           (PE banded matmul)
  gy   = [1,0,-1] along H of hGy           (PE banded matmul)
  sx,sy = Square(psum)                     (ACT, fused psum drain)
Then: gm = sum_c sqrt(sx+sy); gxs/gys via PE on sum_c hGx/hGy;
orientation k from sign/compare algebra; NMS via copy_predicated selects.
"""

import sys

sys.path.insert(0, "/opt/trn_rl_repo")

import numpy as np

import concourse.bass as bass
import concourse.mybir as mybir
from concourse.mybir import AluOpType as Op
from concourse.mybir import ActivationFunctionType as Act
from concourse.tile import TileContext

F32 = mybir.dt.float32

B = 32          # full batch
NCORES = 8
BI = B // NCORES  # images per core
C = 3
H = W = 512
STEP = 124      # output rows per segment
NSEG = 5        # segs 0..3 -> 124 rows each, seg 4 -> 16 rows
SEG = 516       # padded segment width (data in cols 2..513)
PAD = 2

T1 = float(np.float32(np.tan(np.pi / 8)))      # tan(22.5 deg)
T2 = float(np.float32(np.tan(3 * np.pi / 8)))  # tan(67.5 deg)

_g = np.exp(-0.5 * ((np.arange(5) - 2) / 1.0) ** 2).astype(np.float32)
_g = _g / _g.sum()


def _band_stage1():
    """5-tap vertical gaussian lhsT: input partition i <-> row 124j-4+i,
    output partition m <-> row 124j-2+m.  Shape [128, NSEG, 128]."""
    L = np.zeros((128, NSEG, 128), dtype=np.float32)
    for j in range(NSEG):
        for m in range(128):
            go = STEP * j - 2 + m
            if not (0 <= go < H):
                continue
            for i in range(128):
                gi = STEP * j - 4 + i
                d = gi - go
                if 0 <= gi < H and -2 <= d <= 2:
                    L[i, j, m] = _g[d + 2]
    return L


def _band_stage2(w3):
    """3-tap vertical lhsT: input partition i <-> row 124j-2+i,
    output partition m <-> row 124j+m.  Shape [128, NSEG, 124]."""
    L = np.zeros((128, NSEG, 124), dtype=np.float32)
    for j in range(NSEG):
        for m in range(124):
            go = STEP * j + m
            if not (0 <= go < H):
                continue
            for i in range(128):
                gi = STEP * j - 2 + i
                d = gi - go
                if 0 <= gi < H and -1 <= d <= 1:
                    L[i, j, m] = w3[d + 1]
    return L


def _band_stage1_tail():
    """Stage-1 joint tail: input partition t of X seg j+1 <-> row
    124j+120+t; only rows >= 124j+124 (t>=4) are missing from the main
    matmul.  Shape [8, 4, 128]."""
    L = np.zeros((8, 4, 128), dtype=np.float32)
    for j in range(4):
        for m in range(128):
            go = STEP * j - 2 + m
            if not (0 <= go < H):
                continue
            for t in range(4, 8):
                gi = STEP * j + 120 + t
                d = gi - go
                if 0 <= gi < H and -2 <= d <= 2:
                    L[t, j, m] = _g[d + 2]
    return L


L1_NP = _band_stage1()
L1T_NP = _band_stage1_tail()
L2A_NP = _band_stage2(np.array([1.0, 2.0, 1.0], dtype=np.float32))
L2B_NP = _band_stage2(np.array([1.0, 0.0, -1.0], dtype=np.float32))


def _build_program(n_img=BI):
    nc = bass.Bass()

    x_in = nc.dram_tensor("img", [n_img, C, H, W], F32, kind="ExternalInput")
    l1_in = nc.dram_tensor("l1", list(L1_NP.shape), F32, kind="ExternalInput")
    l1t_in = nc.dram_tensor("l1t", list(L1T_NP.shape), F32, kind="ExternalInput")
    l2a_in = nc.dram_tensor("l2a", list(L2A_NP.shape), F32, kind="ExternalInput")
    l2b_in = nc.dram_tensor("l2b", list(L2B_NP.shape), F32, kind="ExternalInput")

    o_blur = nc.dram_tensor("blurred", [n_img, C, H, W], F32, kind="ExternalOutput")
    o_gm = nc.dram_tensor("grad_mag", [n_img, 1, H, W], F32, kind="ExternalOutput")
    o_ori = nc.dram_tensor("grad_ori", [n_img, 1, H, W], F32, kind="ExternalOutput")
    o_thin = nc.dram_tensor("thin", [n_img, 1, H, W], F32, kind="ExternalOutput")
    o_thr = nc.dram_tensor("thresh", [n_img, 1, H, W], F32, kind="ExternalOutput")
    o_early = nc.dram_tensor("early", [n_img, 1, H, W], F32, kind="ExternalOutput")

    import contextlib

    with TileContext(nc) as tc, contextlib.ExitStack() as ctx:
        consts = ctx.enter_context(tc.tile_pool(name="consts", bufs=1))
        work = ctx.enter_context(tc.tile_pool(name="work", bufs=1))
        psum = ctx.enter_context(tc.tile_pool(name="psum", bufs=2, space="PSUM"))

        l1 = consts.tile([128, NSEG, 128], F32)
        l1t = consts.tile([8, 4, 128], F32)
        nc.sync.dma_start(l1t[:], l1t_in[:])
        l2a = consts.tile([128, NSEG, 124], F32)
        l2b = consts.tile([128, NSEG, 124], F32)
        nc.sync.dma_start(l1[:], l1_in[:])
        nc.sync.dma_start(l2a[:], l2a_in[:])
        nc.sync.dma_start(l2b[:], l2b_in[:])

        def padtile(name):
            t = work.tile([128, NSEG, SEG], F32, name=name)
            nc.gpsimd.memset(t[:], 0.0)
            return t

        # 17 map-sized tiles; several are reused under different roles
        X = padtile("X")
        hB = padtile("hB")
        blur = padtile("blur")
        hGx = padtile("hGx")
        hGy = padtile("hGy")
        hGxS = padtile("hGxS")
        hGyS = padtile("hGyS")
        gm = padtile("gm")
        gmu = padtile("gmu")
        gmd = padtile("gmd")
        gxs = padtile("gxs")
        gys = padtile("gys")
        sx = padtile("sx")
        sy = padtile("sy")
        t_r = padtile("t_r")
        zt = padtile("zt")
        qt = padtile("qt")
        U8 = mybir.dt.uint8
        c1u = work.tile([128, NSEG, SEG], U8, name="c1u")
        c2u = work.tile([128, NSEG, SEG], U8, name="c2u")
        cPu = work.tile([128, NSEG, SEG], U8, name="cPu")

        # role aliases (after their primary use is finished)
        c1, c2, cP, cb, mt = X, hB, blur, hGy, hGx
        w2r = t_r
        oo, ot, oh, oe = hGxS, sx, sy, hGyS

        def fa(t, dx=0):
            """X-level AP: all 128 rows (incl. halo), data cols."""
            return t[0:128, :, PAD + dx:PAD + dx + W]

        def va(t, dx=0):
            """valid-map AP: 124 output rows per seg."""
            return t[0:124, :, PAD + dx:PAD + dx + W]

        def body(img):
            # X is aliased as c1 at the end of each image; restore the
            # zero halo partitions that the DMAs below do not rewrite.
            nc.vector.memset(X[0:4, 0, PAD:PAD + W], 0.0)
            nc.gpsimd.memset(X[0:128, 4, PAD:PAD + W], 0.0)
            for ch in range(C):
                src = x_in[img, ch]
                nc.sync.dma_start(X[4:128, 0, PAD:PAD + W], src[0:124, :])
                for j in range(1, 4):
                    nc.sync.dma_start(
                        X[0:128, j, PAD:PAD + W],
                        src[STEP * j - 4:STEP * j + 124, :])
                nc.sync.dma_start(X[0:20, 4, PAD:PAD + W], src[492:512, :])

                # hB = gaussian along W
                g0, g1, g2 = float(_g[0]), float(_g[1]), float(_g[2])
                nc.gpsimd.tensor_tensor(fa(sx), fa(X, -2), fa(X, 2), Op.add)
                nc.vector.tensor_tensor(fa(sy), fa(X, -1), fa(X, 1), Op.add)
                nc.gpsimd.tensor_scalar(fa(hB), fa(X), g2, None, Op.mult)
                nc.vector.scalar_tensor_tensor(
                    fa(hB), fa(sx), g0, fa(hB), Op.mult, Op.add)
                nc.vector.scalar_tensor_tensor(
                    fa(hB), fa(sy), g1, fa(hB), Op.mult, Op.add)

                # blur = gaussian along H (PE)
                for j in range(NSEG):
                    pb = psum.tile([128, W], F32, name="pb", tag="pb")
                    last = j == NSEG - 1
                    nc.tensor.matmul(
                        pb[:], l1[0:128, j, :], hB[0:128, j, PAD:PAD + W],
                        start=True, stop=last)
                    if not last:
                        nc.tensor.matmul(
                            pb[:], l1t[0:8, j, :],
                            hB[0:8, j + 1, PAD:PAD + W],
                            start=False, stop=True)
                    nc.scalar.copy(blur[0:128, j, PAD:PAD + W], pb[:])

                # blurred output (valid rows = partitions 2..125 / 2..17);
                # one DMA per segment keeps each DMA's wait list short
                for j in range(4):
                    nc.sync.dma_start(
                        o_blur[img, ch][STEP * j:STEP * j + STEP, :],
                        blur[2:126, j, PAD:PAD + W])
                nc.sync.dma_start(
                    o_blur[img, ch][496:512, :], blur[2:18, 4, PAD:PAD + W])

                # sobel W-parts
                nc.vector.tensor_tensor(
                    fa(hGx), fa(blur, -1), fa(blur, 1), Op.subtract)
                nc.gpsimd.tensor_tensor(
                    fa(sx), fa(blur, -1), fa(blur, 1), Op.add)
                nc.vector.scalar_tensor_tensor(
                    fa(hGy), fa(blur), 2.0, fa(sx), Op.mult, Op.add)

                if ch == 0:
                    nc.gpsimd.tensor_copy(fa(hGxS), fa(hGx))
                    nc.gpsimd.tensor_copy(fa(hGyS), fa(hGy))
                else:
                    nc.gpsimd.tensor_tensor(fa(hGxS), fa(hGxS), fa(hGx), Op.add)
                    nc.gpsimd.tensor_tensor(fa(hGyS), fa(hGyS), fa(hGy), Op.add)

                # sobel H-parts (PE) with fused square drains
                for j in range(NSEG):
                    pgx = psum.tile([124, W], F32, name="pgx", tag="pgx")
                    pgy = psum.tile([124, W], F32, name="pgy", tag="pgy")
                    nc.tensor.matmul(
                        pgx[:], l2a[0:128, j, 0:124],
                        hGx[0:128, j, PAD:PAD + W], start=True, stop=True)
                    nc.tensor.matmul(
                        pgy[:], l2b[0:128, j, 0:124],
                        hGy[0:128, j, PAD:PAD + W], start=True, stop=True)
                    nc.scalar.square(sx[0:124, j, PAD:PAD + W], pgx[:])
                    nc.scalar.square(sy[0:124, j, PAD:PAD + W], pgy[:])

                nc.vector.tensor_tensor(va(sx), va(sx), va(sy), Op.add)
                if ch == 0:
                    nc.scalar.sqrt(va(gm), va(sx))
                else:
                    nc.scalar.sqrt(va(t_r), va(sx))
                    nc.vector.tensor_tensor(va(gm), va(gm), va(t_r), Op.add)

            # gxs / gys via PE on channel sums
            for j in range(NSEG):
                pgx = psum.tile([124, W], F32, name="pgx2", tag="pgx")
                pgy = psum.tile([124, W], F32, name="pgy2", tag="pgy")
                nc.tensor.matmul(
                    pgx[:], l2a[0:128, j, 0:124],
                    hGxS[0:128, j, PAD:PAD + W], start=True, stop=True)
                nc.tensor.matmul(
                    pgy[:], l2b[0:128, j, 0:124],
                    hGyS[0:128, j, PAD:PAD + W], start=True, stop=True)
                nc.scalar.copy(gxs[0:124, j, PAD:PAD + W], pgx[:])
                nc.scalar.copy(gys[0:124, j, PAD:PAD + W], pgy[:])

            # gm row-shifted copies (gmu = row+1, gmd = row-1)
            nc.sync.dma_start(
                gmu[0:123, :, PAD:PAD + W], gm[1:124, :, PAD:PAD + W])
            nc.sync.dma_start(
                gmu[123:124, 0:4, PAD:PAD + W], gm[0:1, 1:5, PAD:PAD + W])
            nc.sync.dma_start(
                gmd[1:124, :, PAD:PAD + W], gm[0:123, :, PAD:PAD + W])
            nc.sync.dma_start(
                gmd[0:1, 1:5, PAD:PAD + W], gm[123:124, 0:4, PAD:PAD + W])

            # orientation masks: c1 = t1|a|>|b|, c2 = t2|a|<|b|,
            # cP = a*b>0, cb = b>0
            nc.scalar.activation(va(sx), va(gxs), Act.Abs)
            nc.scalar.activation(va(sy), va(gys), Act.Abs)
            nc.vector.scalar_tensor_tensor(
                va(c1), va(sx), T1, va(sy), Op.mult, Op.is_gt)
            nc.vector.scalar_tensor_tensor(
                va(c2), va(sx), T2, va(sy), Op.mult, Op.is_lt)
            nc.vector.tensor_tensor(va(zt), va(gxs), va(gys), Op.mult)
            nc.vector.tensor_scalar(va(cP), va(zt), 0.0, None, Op.is_gt)
            nc.gpsimd.tensor_scalar(va(cb), va(gys), 0.0, None, Op.is_gt)
            # uint8 mask copies for CopyPredicated (requires int mask dtype)
            nc.vector.tensor_copy(va(c1u), va(c1))
            nc.vector.tensor_copy(va(c2u), va(c2))
            nc.gpsimd.tensor_copy(va(cPu), va(cP))

            # ori = 45*k' + 135, k' = w2r + 4cb - 2*cP*(w2r+1)
            nc.vector.tensor_tensor(va(w2r), va(c1), va(c2), Op.subtract)
            nc.vector.scalar_tensor_tensor(
                va(zt), va(w2r), 1.0, va(cP), Op.add, Op.mult)
            nc.vector.scalar_tensor_tensor(
                va(qt), va(cb), 4.0, va(w2r), Op.mult, Op.add)
            nc.vector.scalar_tensor_tensor(
                va(zt), va(zt), -2.0, va(qt), Op.mult, Op.add)
            nc.vector.tensor_scalar(va(oo), va(zt), 45.0, 135.0, Op.mult, Op.add)
            nc.sync.dma_start(
                o_ori[img, 0][0:496, :].rearrange("(s p) w -> p s w", p=STEP),
                oo[0:124, 0:4, PAD:PAD + W])
            nc.sync.dma_start(
                o_ori[img, 0][496:512, :], oo[0:16, 4, PAD:PAD + W])

            # NMS: Mn(qt) built by priority-ordered predicated overwrites
            nc.vector.tensor_tensor(va(mt), va(gmu, -1), va(gmd, 1), Op.max)
            nc.gpsimd.tensor_copy(va(qt), va(mt))                    # m3
            nc.vector.tensor_tensor(va(mt), va(gmu, 1), va(gmd, -1), Op.max)
            nc.vector.copy_predicated(va(qt), va(cPu), va(mt))       # m1
            nc.vector.tensor_tensor(va(mt), va(gmu), va(gmd), Op.max)
            nc.vector.copy_predicated(va(qt), va(c2u), va(mt))       # m2
            nc.vector.tensor_tensor(va(mt), va(gm, 1), va(gm, -1), Op.max)
            nc.vector.copy_predicated(va(qt), va(c1u), va(mt))       # m0
            nc.vector.tensor_tensor(va(zt), va(gm), va(qt), Op.is_gt)  # im

            # early = gm*(gm>=2); thin = gm*im; thresh = im*early
            nc.gpsimd.tensor_scalar(va(w2r), va(gm), 2.0, None, Op.is_ge)
            nc.vector.tensor_tensor(va(oe), va(gm), va(w2r), Op.mult)
            nc.vector.tensor_tensor(va(ot), va(gm), va(zt), Op.mult)
            nc.vector.tensor_tensor(va(oh), va(zt), va(oe), Op.mult)

            for t, dst in ((gm, o_gm), (ot, o_thin), (oh, o_thr),
                           (oe, o_early)):
                nc.sync.dma_start(
                    dst[img, 0][0:496, :].rearrange("(s p) w -> p s w", p=STEP),
                    t[0:124, 0:4, PAD:PAD + W])
                nc.sync.dma_start(
                    dst[img, 0][496:512, :], t[0:16, 4, PAD:PAD + W])

        for img in range(n_img):
            body(img)

    return nc


def _split_multi_waits(nc):
    """This toolchain's walrus allows at most one attached sync-wait per
    instruction.  Hoist extra waits into preceding single-wait NoOps on the
    same engine (sequencers execute them in order, so semantics are kept)."""
    import orjson

    d = orjson.loads(nc.to_json_bytes())
    uid = [0]
    for fn in d["functions"]:
        for bb in fn["blocks"]:
            out = []
            for ins in bb["instructions"]:
                si = ins.get("sync_info")
                waits = si.get("on_wait", []) if si else []
                if len(waits) > 1:
                    for wcond in waits[:-1]:
                        uid[0] += 1
                        out.append({
                            "name": f"WSPLIT-{uid[0]}",
                            "opcode": "NoOp",
                            "engine": ins["engine"],
                            "ins": [],
                            "outs": [],
                            "sync_info": {"on_update": [],
                                          "on_wait": [wcond]},
                        })
                    si["on_wait"] = [waits[-1]]
                out.append(ins)
            bb["instructions"] = out
    nc.m = mybir.parse_bytes(orjson.dumps(d))
    return nc


_PROGRAM = None


def _get_program():
    global _PROGRAM
    if _PROGRAM is None:
        _PROGRAM = _split_multi_waits(_build_program())
    return _PROGRAM


def kernel(img):
    from concourse.bass_utils import run_bass_kernel_spmd

    img = np.asarray(img, dtype=np.float32)
    nc = _get_program()
    in_maps = []
    for c in range(NCORES):
        in_maps.append({
            "img": np.ascontiguousarray(img[c * BI:(c + 1) * BI]),
            "l1": L1_NP, "l1t": L1T_NP, "l2a": L2A_NP, "l2b": L2B_NP,
        })
    res = run_bass_kernel_spmd(nc, in_maps, list(range(NCORES))).results

    def cat(name):
        return np.concatenate([res[c][name] for c in range(NCORES)], axis=0)

    return (cat("blurred"), cat("grad_mag"), cat("grad_ori"),
            cat("thin"), cat("thresh"), cat("early"))
